# revision 1
# baseline (speedup 1.0000x reference)
"""Trainium2 Bass kernel for nn_CoeffNet (gnn_message_passing).

Strategy (8 NeuronCores, SPMD):
  - Nodes range-sharded by dst: core c owns nodes [c*NPC, (c+1)*NPC).
  - Edges sharded by dst range, sorted by dst, padded so every 128-node
    block owns a fixed number of edge tiles (static program).
  - Per refinement: dma_gather a = x[src] from a bf16 HBM table, PE
    matmul projects the extended basis [rbf | r1*rbf | r2*rbf | r3*rbf]
    through W_ext to produce all rhat-scaled variants, DVE forms the
    CG-product messages in bf16, PE one-hot matmuls segment-sum the
    messages into PSUM per node block, fp32 x lives in SBUF, AllGather
    rebuilds the bf16 table between refinements.
  - Output dense layer computed on-device per core; host concatenates.

kernel(**inputs) takes the FULL unsharded inputs and returns the FULL
(N, 4, 1) float32 output.
"""

import os
import sys

import numpy as np

for _p in ("/opt/trn_rl_repo", "/root/.axon_site/_ro/trn_rl_repo"):
    if os.path.isdir(_p) and _p not in sys.path:
        sys.path.insert(0, _p)


class CFG:
    """Problem/layout configuration (class attrs so tests can shrink it)."""
    N_NODES = 20000
    F = 64
    KB = 32            # radial basis functions (4*KB must be 128)
    R_REF = 3
    R_MAX = 2.5
    N_CORES = 8
    NPC = 2500         # real nodes per core
    NBLK = 20          # 128-node blocks per core
    TPB = 18           # edge tiles per node block (static capacity)
    TSUB = 6           # tiles per sub-slab (DVE op batching)

    @classmethod
    def derived(cls):
        c = cls
        c.NLOC = c.NBLK * 128
        c.TAB_STRIDE = c.NLOC
        c.NTAB = c.TAB_STRIDE * c.N_CORES
        c.EPB = c.TPB * 128
        c.NT = c.NBLK * c.TPB
        c.EPAD = c.NT * 128
        c.NSUB = c.TPB // c.TSUB
        c.ROW = 4 * c.F
        c.VCOLS = 14 * c.F           # A|rB(3)|rC(3)|D|rEA(3)|rEBn(3)
        c.C_A = 0
        c.C_RB = c.F
        c.C_RC = 4 * c.F
        c.C_D = 7 * c.F
        c.C_REA = 8 * c.F
        c.C_REB = 11 * c.F
        assert c.TPB % c.TSUB == 0
        return c


CFG.derived()


# ---------------------------------------------------------------- host prep
def _host_geometry(coords, dst_idx, src_idx):
    c = CFG
    r = coords[dst_idx] - coords[src_idx]
    d = np.sqrt(np.sum(r * r, axis=-1) + 1e-12).astype(np.float32)
    rhat = (r / d[:, None]).astype(np.float32)
    centers = np.linspace(0.0, c.R_MAX, c.KB, dtype=np.float32)
    width = centers[1] - centers[0]
    rbf = np.exp(-(((d[:, None] - centers) / width) ** 2)).astype(np.float32)
    return rhat, rbf


def _host_shard(dst_idx, src_idx, rhat, rbf):
    c = CFG
    order = np.argsort(dst_idx, kind="stable")
    dst_s = dst_idx[order]
    src_s = src_idx[order]
    rhat_s = rhat[order]
    rbf_s = rbf[order]

    cores = []
    core_starts = np.searchsorted(dst_s, np.arange(0, c.N_NODES + 1, c.NPC))
    for ci in range(c.N_CORES):
        lo, hi = core_starts[ci], core_starts[ci + 1]
        dst_c = dst_s[lo:hi] - ci * c.NPC
        src_c = src_s[lo:hi]
        rhat_c = rhat_s[lo:hi]
        rbf_c = rbf_s[lo:hi]

        blk = dst_c >> 7
        blk_starts = np.searchsorted(blk, np.arange(c.NBLK + 1))

        src_pad = np.zeros(c.EPAD, np.int32)
        dstr_pad = np.full(c.EPAD, -1.0, np.float32)
        rhat_pad = np.zeros((c.EPAD, 3), np.float32)
        rbf_pad = np.zeros((c.EPAD, c.KB), np.float32)
        for b in range(c.NBLK):
            s, e = blk_starts[b], blk_starts[b + 1]
            n = e - s
            if n > c.EPB:
                raise RuntimeError(
                    f"core {ci} block {b}: {n} edges > capacity {c.EPB}"
                )
            o = b * c.EPB
            src_pad[o:o + n] = src_c[s:e]
            dstr_pad[o:o + n] = (dst_c[s:e] & 127).astype(np.float32)
            rhat_pad[o:o + n] = rhat_c[s:e]
            rbf_pad[o:o + n] = rbf_c[s:e]

        sc = src_pad // c.NPC
        su = src_pad - sc * c.NPC
        tab_row = (sc * c.TAB_STRIDE + su).astype(np.int16)

        # [16, EPAD//16] wrapped (idx j at [j%16, j//16]) then replicated
        # to all 128 partitions (8 Q7 cores each read their own group).
        srcw16 = np.ascontiguousarray(tab_row.reshape(c.EPAD // 16, 16).T)
        srcw = np.tile(srcw16, (8, 1))

        dstr_w = np.ascontiguousarray(dstr_pad.reshape(c.NT, 128).T)

        basis = np.concatenate(
            [rbf_pad,
             rhat_pad[:, 0:1] * rbf_pad,
             rhat_pad[:, 1:2] * rbf_pad,
             rhat_pad[:, 2:3] * rbf_pad], axis=1
        )
        basisT = np.ascontiguousarray(
            basis.reshape(c.NT, 128, 4 * c.KB).transpose(0, 2, 1)
        ).astype(np.float32)

        cores.append(dict(srcw=srcw, dstr=dstr_w, basisT=basisT))
    return cores


def _host_weights(W_basis, b_basis, path_w):
    c = CFG
    F = c.F
    Wx = np.zeros((c.R_REF, 4 * c.KB, c.VCOLS), np.float32)
    brep = np.zeros((c.R_REF, 2, F), np.float32)
    for r in range(c.R_REF):
        W = W_basis[r]
        p = path_w[r]
        b = b_basis[r]
        Wx[r, 0:c.KB, c.C_A:c.C_A + F] = W * p[0]
        Wx[r, 0:c.KB, c.C_D:c.C_D + F] = W * p[3]
        for j in range(3):
            rows = slice((j + 1) * c.KB, (j + 2) * c.KB)
            Wx[r, rows, c.C_RB + j * F:c.C_RB + (j + 1) * F] = W * p[1]
            Wx[r, rows, c.C_RC + j * F:c.C_RC + (j + 1) * F] = W * p[2]
        for j in range(3):
            ra = (j + 1) % 3
            rb = (j + 2) % 3
            Wx[r, (ra + 1) * c.KB:(ra + 2) * c.KB,
               c.C_REA + j * F:c.C_REA + (j + 1) * F] = W * p[4]
            Wx[r, (rb + 1) * c.KB:(rb + 2) * c.KB,
               c.C_REB + j * F:c.C_REB + (j + 1) * F] = -(W * p[4])
        brep[r, 0] = p[0] * b
        brep[r, 1] = p[3] * b
    return Wx, brep


def _tensor_dense_np(x, W0, W1, b0):
    s = x[:, 0] @ W0 + b0
    v = np.einsum("nif,fg->nig", x[:, 1:4], W1)
    return np.concatenate([s[:, None], v], axis=1)


# ---------------------------------------------------------------- bass build
_BUILD_CACHE = {}


def build(with_bias):
    key = (bool(with_bias), CFG.N_CORES, CFG.NBLK, CFG.TPB)
    if key in _BUILD_CACHE:
        return _BUILD_CACHE[key]

    import concourse.bacc as bacc
    import concourse.mybir as mybir
    import concourse.tile as tile
    from concourse import library_config
    from concourse.alu_op_type import AluOpType
    dt = mybir.dt
    c = CFG
    F = c.F
    ROW = c.ROW

    nc = bacc.Bacc(num_devices=c.N_CORES, target_bir_lowering=False)

    x0f_d = nc.dram_tensor("x0f", [c.NBLK, 128, ROW], dt.float32, kind="ExternalInput")
    xtab0_d = nc.dram_tensor("xtab0", [c.NTAB, ROW], dt.bfloat16, kind="ExternalInput")
    srcw_d = nc.dram_tensor("srcw", [128, c.EPAD // 16], dt.int16, kind="ExternalInput")
    dstr_d = nc.dram_tensor("dstr", [128, c.NT], dt.bfloat16, kind="ExternalInput")
    basis_d = nc.dram_tensor("basisT", [c.NT, 4 * c.KB, 128], dt.bfloat16, kind="ExternalInput")
    wext_d = nc.dram_tensor("wext", [c.R_REF, 4 * c.KB, c.VCOLS], dt.bfloat16, kind="ExternalInput")
    brep_d = nc.dram_tensor("brep", [c.R_REF, 2, 128, F], dt.bfloat16, kind="ExternalInput")
    iota_d = nc.dram_tensor("iotar", [128, 128], dt.bfloat16, kind="ExternalInput")
    wout_d = nc.dram_tensor("woutrep", [128, ROW], dt.float32, kind="ExternalInput")
    out_d = nc.dram_tensor("out", [c.NBLK, 128, 4], dt.float32, kind="ExternalOutput")

    xtab1_d = nc.dram_tensor("xtab1", [c.NTAB, ROW], dt.bfloat16, addr_space="Shared")
    xtab2_d = nc.dram_tensor("xtab2", [c.NTAB, ROW], dt.bfloat16, addr_space="Shared")
    xslice_d = nc.dram_tensor("xslice", [c.NLOC, ROW], dt.bfloat16)

    tabs = [xtab0_d, xtab1_d, xtab2_d]

    with tile.TileContext(nc) as tc:
        with (
            tc.tile_pool(name="resident", bufs=1) as res,
            tc.tile_pool(name="apool", bufs=3) as apool,
            tc.tile_pool(name="vpsum", bufs=2, space="PSUM") as vpsum,
            tc.tile_pool(name="spsum", bufs=2, space="PSUM") as spsum,
            tc.tile_pool(name="vbfp", bufs=2) as vbfp,
            tc.tile_pool(name="msgp", bufs=2) as msgp,
            tc.tile_pool(name="scr", bufs=2) as scr,
            tc.tile_pool(name="basp", bufs=3) as basp,
        ):
            xf32 = res.tile([128, c.NBLK, ROW], dt.float32)
            srcw = res.tile([128, c.EPAD // 16], dt.int16)
            dstr = res.tile([128, c.NT], dt.bfloat16)
            iota = res.tile([128, 128], dt.bfloat16)
            wext = res.tile([4 * c.KB, c.R_REF, c.VCOLS], dt.bfloat16)
            brepA = res.tile([128, c.R_REF, F], dt.bfloat16)
            brepD = res.tile([128, c.R_REF, F], dt.bfloat16)
            woutr = res.tile([128, ROW], dt.float32)
            outsb = res.tile([128, c.NBLK, 4], dt.float32)

            nc.sync.dma_start(xf32[:], x0f_d[:].rearrange("b p f -> p b f"))
            nc.sync.dma_start(srcw[:], srcw_d[:])
            nc.sync.dma_start(dstr[:], dstr_d[:])
            nc.sync.dma_start(iota[:], iota_d[:])
            nc.sync.dma_start(wext[:], wext_d[:].rearrange("r p col -> p r col"))
            nc.sync.dma_start(brepA[:], brep_d[:, 0].rearrange("r p f -> p r f"))
            nc.sync.dma_start(brepD[:], brep_d[:, 1].rearrange("r p f -> p r f"))
            nc.sync.dma_start(woutr[:], wout_d[:])

            nc.gpsimd.load_library(library_config.mlp)

            TT = nc.vector.tensor_tensor

            for r in range(c.R_REF):
                tab = tabs[r]
                for b in range(c.NBLK):
                    psB = spsum.tile([128, ROW], dt.float32, tag="psB")
                    for ss in range(c.NSUB):
                        t0 = b * c.TPB + ss * c.TSUB
                        e0 = t0 * 128
                        ne = c.TSUB * 128
                        a = apool.tile([128, c.TSUB, ROW], dt.bfloat16, tag="a")
                        nc.gpsimd.dma_gather(
                            a[:],
                            tab[:],
                            srcw[:, e0 // 16:(e0 + ne) // 16],
                            ne,
                            ne,
                            ROW,
                        )
                        bas = basp.tile([4 * c.KB, c.TSUB, 128], dt.bfloat16, tag="bas")
                        nc.sync.dma_start(
                            bas[:],
                            basis_d[t0:t0 + c.TSUB].rearrange("t p e -> p t e"),
                        )
                        vbf = vbfp.tile([128, c.TSUB, c.VCOLS], dt.bfloat16, tag="vbf")
                        for t in range(c.TSUB):
                            vps = vpsum.tile([128, c.VCOLS], dt.float32, tag="vps")
                            for lo in range(0, c.VCOLS, 512):
                                hi = min(lo + 512, c.VCOLS)
                                nc.tensor.matmul(
                                    vps[:, lo:hi], bas[:, t, :],
                                    wext[:, r, lo:hi],
                                    start=True, stop=True,
                                )
                            nc.scalar.activation(
                                vbf[:, t, :], vps[:],
                                mybir.ActivationFunctionType.Copy,
                            )
                        if with_bias:
                            TT(vbf[:, :, c.C_A:c.C_A + F],
                               vbf[:, :, c.C_A:c.C_A + F],
                               brepA[:, r, :].unsqueeze(1).to_broadcast(
                                   [128, c.TSUB, F]),
                               op=AluOpType.add)
                            TT(vbf[:, :, c.C_D:c.C_D + F],
                               vbf[:, :, c.C_D:c.C_D + F],
                               brepD[:, r, :].unsqueeze(1).to_broadcast(
                                   [128, c.TSUB, F]),
                               op=AluOpType.add)

                        a0 = a[:, :, 0:F]
                        av = a[:, :, F:ROW]
                        msg = msgp.tile([128, c.TSUB, ROW], dt.bfloat16, tag="msg")
                        m0 = msg[:, :, 0:F]
                        mv = msg[:, :, F:ROW]
                        t2 = scr.tile([128, c.TSUB, 3 * F], dt.bfloat16, tag="t2")
                        t4 = scr.tile([128, c.TSUB, 3 * F], dt.bfloat16, tag="t4")
                        q5 = scr.tile([128, c.TSUB, 3 * F], dt.bfloat16, tag="q5")
                        q6 = scr.tile([128, c.TSUB, 3 * F], dt.bfloat16, tag="q6")
                        S = scr.tile([128, c.TSUB, 128], dt.bfloat16, tag="S")

                        # m0 = a0*A + sum_j av_j*(r_j*p1G)
                        TT(m0, a0, vbf[:, :, c.C_A:c.C_A + F], op=AluOpType.mult)
                        TT(t2[:], av, vbf[:, :, c.C_RB:c.C_RB + 3 * F],
                           op=AluOpType.mult)
                        for j in range(3):
                            TT(m0, m0, t2[:, :, j * F:(j + 1) * F],
                               op=AluOpType.add)
                        # mv_i = a0*(r_i*p2G) + av_i*(p3G) + cross-terms
                        TT(mv.rearrange("p t (cc f) -> p t cc f", cc=3),
                           a0.unsqueeze(2).to_broadcast([128, c.TSUB, 3, F]),
                           vbf[:, :, c.C_RC:c.C_RC + 3 * F].rearrange(
                               "p t (cc f) -> p t cc f", cc=3),
                           op=AluOpType.mult)
                        TT(t4[:].rearrange("p t (cc f) -> p t cc f", cc=3),
                           av.rearrange("p t (cc f) -> p t cc f", cc=3),
                           vbf[:, :, c.C_D:c.C_D + F].unsqueeze(2).to_broadcast(
                               [128, c.TSUB, 3, F]),
                           op=AluOpType.mult)
                        TT(mv, mv, t4[:], op=AluOpType.add)
                        TT(q5[:], av, vbf[:, :, c.C_REA:c.C_REA + 3 * F],
                           op=AluOpType.mult)
                        TT(q6[:], av, vbf[:, :, c.C_REB:c.C_REB + 3 * F],
                           op=AluOpType.mult)
                        # q5_j = av_j*r_{j+1}*p4G -> mv_{j-1} (rotate left)
                        TT(mv[:, :, 0:2 * F], mv[:, :, 0:2 * F],
                           q5[:, :, F:3 * F], op=AluOpType.add)
                        TT(mv[:, :, 2 * F:3 * F], mv[:, :, 2 * F:3 * F],
                           q5[:, :, 0:F], op=AluOpType.add)
                        # q6_j = -av_j*r_{j+2}*p4G -> mv_{j+1} (rotate right)
                        TT(mv[:, :, F:3 * F], mv[:, :, F:3 * F],
                           q6[:, :, 0:2 * F], op=AluOpType.add)
                        TT(mv[:, :, 0:F], mv[:, :, 0:F],
                           q6[:, :, 2 * F:3 * F], op=AluOpType.add)

                        TT(S[:],
                           iota[:].unsqueeze(1).to_broadcast([128, c.TSUB, 128]),
                           dstr[:, t0:t0 + c.TSUB].unsqueeze(2).to_broadcast(
                               [128, c.TSUB, 128]),
                           op=AluOpType.is_equal)

                        for t in range(c.TSUB):
                            nc.tensor.matmul(
                                psB[:], S[:, t, :], msg[:, t, :],
                                start=(ss == 0 and t == 0),
                                stop=(ss == c.NSUB - 1 and t == c.TSUB - 1),
                            )
                    TT(xf32[:, b, :], xf32[:, b, :], psB[:], op=AluOpType.add)
                if r < c.R_REF - 1:
                    nc.gpsimd.dma_start(
                        xslice_d[:].rearrange("(b p) f -> p b f", p=128),
                        xf32[:],
                    )
                    nc.gpsimd.collective_compute(
                        "AllGather",
                        mybir.AluOpType.bypass,
                        replica_groups=[list(range(c.N_CORES))],
                        ins=[xslice_d[:]],
                        outs=[tabs[r + 1][:]],
                    )

            for b in range(c.NBLK):
                tq = scr.tile([128, ROW], dt.float32, tag="tq")
                TT(tq[:], xf32[:, b, :], woutr[:], op=AluOpType.mult)
                nc.vector.tensor_reduce(
                    outsb[:, b, :].unsqueeze(-1),
                    tq[:].rearrange("p (l f) -> p l f", l=4),
                    axis=mybir.AxisListType.X,
                    op=AluOpType.add,
                )
            nc.sync.dma_start(out_d[:].rearrange("b p l -> p b l"), outsb[:])

    nc.compile()
    _BUILD_CACHE[key] = nc
    return nc


# ---------------------------------------------------------------- runner
def prep_in_maps(inputs):
    """Host preprocessing -> (in_maps, with_bias)."""
    import ml_dtypes
    c = CFG
    bf16 = ml_dtypes.bfloat16

    x_dftb = np.asarray(inputs["x_dftb"], np.float32)
    coords = np.asarray(inputs["coords"], np.float32)
    dst_idx = np.asarray(inputs["dst_idx"], np.int32)
    src_idx = np.asarray(inputs["src_idx"], np.int32)
    W_in0 = np.asarray(inputs["W_in0"], np.float32)
    W_in1 = np.asarray(inputs["W_in1"], np.float32)
    b_in = np.asarray(inputs["b_in"], np.float32)
    W_basis = np.asarray(inputs["W_basis"], np.float32)
    b_basis = np.asarray(inputs["b_basis"], np.float32)
    path_w = np.asarray(inputs["path_w"], np.float32)
    W_out0 = np.asarray(inputs["W_out0"], np.float32)
    W_out1 = np.asarray(inputs["W_out1"], np.float32)

    rhat, rbf = _host_geometry(coords, dst_idx, src_idx)
    shards = _host_shard(dst_idx, src_idx, rhat, rbf)
    Wx, brep = _host_weights(W_basis, b_basis, path_w)

    x0 = _tensor_dense_np(x_dftb, W_in0, W_in1, b_in)
    x0_rows = x0.reshape(c.N_NODES, c.ROW)

    tab0 = np.zeros((c.NTAB, c.ROW), np.float32)
    x0fs = []
    for ci in range(c.N_CORES):
        x0c = np.zeros((c.NLOC, c.ROW), np.float32)
        x0c[:c.NPC] = x0_rows[ci * c.NPC:(ci + 1) * c.NPC]
        tab0[ci * c.TAB_STRIDE:ci * c.TAB_STRIDE + c.NLOC] = x0c
        x0fs.append(x0c)

    iota = np.broadcast_to(
        np.arange(128, dtype=np.float32)[None, :], (128, 128)).copy()
    wout = np.concatenate(
        [W_out0[:, 0], W_out1[:, 0], W_out1[:, 0], W_out1[:, 0]]
    ).astype(np.float32)
    wout_rep = np.ascontiguousarray(
        np.broadcast_to(wout[None, :], (128, c.ROW)))

    tab0_bf = tab0.astype(bf16)
    wext_bf = Wx.astype(bf16)
    brep_bf = np.ascontiguousarray(
        np.broadcast_to(brep[:, :, None, :], (c.R_REF, 2, 128, c.F))
    ).astype(bf16)
    iota_bf = iota.astype(bf16)

    in_maps = []
    for ci in range(c.N_CORES):
        sh = shards[ci]
        in_maps.append(dict(
            x0f=np.ascontiguousarray(x0fs[ci].reshape(c.NBLK, 128, c.ROW)),
            xtab0=tab0_bf,
            srcw=sh["srcw"],
            dstr=sh["dstr"].astype(bf16),
            basisT=sh["basisT"].astype(bf16),
            wext=wext_bf,
            brep=brep_bf,
            iotar=iota_bf,
            woutrep=wout_rep,
        ))
    with_bias = bool(np.any(b_basis != 0.0))
    return in_maps, with_bias


def assemble_output(results, b_out):
    c = CFG
    outs = []
    for ci in range(c.N_CORES):
        o = np.asarray(results[ci]["out"]).reshape(c.NLOC, 4)[:c.NPC]
        outs.append(o)
    out = np.concatenate(outs, axis=0).reshape(c.N_NODES, 4, 1)
    out = out.astype(np.float32).copy()
    out[:, 0, :] += np.asarray(b_out, np.float32)[None, :]
    return out


LAST_RESULTS = None


def kernel(**inputs):
    global LAST_RESULTS
    from concourse.bass_utils import run_bass_kernel_spmd

    in_maps, with_bias = prep_in_maps(inputs)
    nc = build(with_bias)
    trace = bool(os.environ.get("KERNEL_TRACE"))
    res = run_bass_kernel_spmd(
        nc, in_maps, core_ids=list(range(CFG.N_CORES)), trace=trace)
    LAST_RESULTS = res
    return assemble_output(res.results, inputs["b_out"])



# revision 3
# speedup vs baseline: 1.4183x; 1.4183x over previous
"""Trainium2 Bass kernel for nn_CoeffNet (gnn_message_passing).

Strategy (8 NeuronCores, SPMD):
  - Nodes range-sharded by dst: core c owns nodes [c*NPC, (c+1)*NPC).
  - Edges sharded by dst range, sorted by dst, padded so every 128-node
    block owns a fixed number of edge tiles (static program).
  - Per refinement: dma_gather a = x[src] from a bf16 HBM table, PE
    matmul projects the extended basis [rbf | r1*rbf | r2*rbf | r3*rbf]
    through W_ext (cols [A|RB|RC|D|U], 11F) to produce the per-edge
    coefficient vectors, DVE forms only raw products in bf16 (cross
    rotations are deferred), PE one-hot matmuls (S precomputed on the
    host) segment-sum the 640-wide product columns into PSUM per node
    block, and a small per-block fp32 "fold" applies the rotated
    cross-product column mapping while accumulating into resident x.
  - PE issue order puts the next sub-slab's projection ahead of the
    current sub-slab's scatter matmul so the DVE chain never stalls.
  - fp32 x lives in SBUF, AllGather rebuilds the bf16 table between
    refinements. Output dense layer computed on-device per core.

kernel(**inputs) takes the FULL unsharded inputs and returns the FULL
(N, 4, 1) float32 output.
"""

import os
import sys

import numpy as np

for _p in ("/opt/trn_rl_repo", "/root/.axon_site/_ro/trn_rl_repo"):
    if os.path.isdir(_p) and _p not in sys.path:
        sys.path.insert(0, _p)


class CFG:
    """Problem/layout configuration (class attrs so tests can shrink it)."""
    N_NODES = 20000
    F = 64
    KB = 32            # radial basis functions (4*KB must be 128)
    R_REF = 3
    R_MAX = 2.5
    N_CORES = 8
    NPC = 2500         # real nodes per core
    NBLK = 20          # 128-node blocks per core
    TPB = 18           # edge tiles per node block (static capacity)
    TSUB = 6           # tiles per sub-slab (DVE op batching)

    @classmethod
    def derived(cls):
        c = cls
        c.NLOC = c.NBLK * 128
        c.TAB_STRIDE = c.NLOC
        c.NTAB = c.TAB_STRIDE * c.N_CORES
        c.EPB = c.TPB * 128
        c.NT = c.NBLK * c.TPB
        c.EPAD = c.NT * 128
        c.NSUB = c.TPB // c.TSUB
        c.ROW = 4 * c.F
        # wext columns: A | RB(3) | RC(3) | D | U(3)  -> 11F
        c.VCOLS = 11 * c.F
        c.C_A = 0
        c.C_RB = c.F
        c.C_RC = 4 * c.F
        c.C_D = 7 * c.F
        c.C_U = 8 * c.F
        # msg columns: m0 | mv(3) | q5(3) | q6(3) -> 10F
        c.MCOLS = 10 * c.F
        c.M_M0 = 0
        c.M_MV = c.F
        c.M_Q5 = 4 * c.F
        c.M_Q6 = 7 * c.F
        assert c.TPB % c.TSUB == 0
        return c


CFG.derived()


# ---------------------------------------------------------------- host prep
def _host_geometry(coords, dst_idx, src_idx):
    c = CFG
    r = coords[dst_idx] - coords[src_idx]
    d = np.sqrt(np.sum(r * r, axis=-1) + 1e-12).astype(np.float32)
    rhat = (r / d[:, None]).astype(np.float32)
    centers = np.linspace(0.0, c.R_MAX, c.KB, dtype=np.float32)
    width = centers[1] - centers[0]
    rbf = np.exp(-(((d[:, None] - centers) / width) ** 2)).astype(np.float32)
    return rhat, rbf


def _host_shard(dst_idx, src_idx, rhat, rbf):
    c = CFG
    order = np.argsort(dst_idx, kind="stable")
    dst_s = dst_idx[order]
    src_s = src_idx[order]
    rhat_s = rhat[order]
    rbf_s = rbf[order]

    cores = []
    core_starts = np.searchsorted(dst_s, np.arange(0, c.N_NODES + 1, c.NPC))
    for ci in range(c.N_CORES):
        lo, hi = core_starts[ci], core_starts[ci + 1]
        dst_c = dst_s[lo:hi] - ci * c.NPC
        src_c = src_s[lo:hi]
        rhat_c = rhat_s[lo:hi]
        rbf_c = rbf_s[lo:hi]

        blk = dst_c >> 7
        blk_starts = np.searchsorted(blk, np.arange(c.NBLK + 1))

        src_pad = np.zeros(c.EPAD, np.int32)
        dstr_pad = np.full(c.EPAD, -1, np.int32)
        rhat_pad = np.zeros((c.EPAD, 3), np.float32)
        rbf_pad = np.zeros((c.EPAD, c.KB), np.float32)
        for b in range(c.NBLK):
            s, e = blk_starts[b], blk_starts[b + 1]
            n = e - s
            if n > c.EPB:
                raise RuntimeError(
                    f"core {ci} block {b}: {n} edges > capacity {c.EPB}"
                )
            o = b * c.EPB
            src_pad[o:o + n] = src_c[s:e]
            dstr_pad[o:o + n] = dst_c[s:e] & 127
            rhat_pad[o:o + n] = rhat_c[s:e]
            rbf_pad[o:o + n] = rbf_c[s:e]

        sc = src_pad // c.NPC
        su = src_pad - sc * c.NPC
        tab_row = (sc * c.TAB_STRIDE + su).astype(np.int16)

        # [16, EPAD//16] wrapped (idx j at [j%16, j//16]) then replicated
        # to all 128 partitions (8 Q7 cores each read their own group).
        srcw16 = np.ascontiguousarray(tab_row.reshape(c.EPAD // 16, 16).T)
        srcw = np.tile(srcw16, (8, 1))

        # one-hot scatter matrix, wrapped: edge (p, t) = padded index
        # t*128+p; S[p, t*128+n] = (dst_slot == n). Pad rows (-1) are all
        # zero.  Layout [128, NT*128] = per-partition contiguous for DMA.
        dstr_w = np.ascontiguousarray(dstr_pad.reshape(c.NT, 128).T)  # [128, NT]
        S = (dstr_w[:, :, None] == np.arange(128, dtype=np.int32)[None, None, :])
        S = np.ascontiguousarray(S.reshape(128, c.NT * 128)).astype(np.float32)

        # extended basis rows: [rbf | r0*rbf | r1*rbf | r2*rbf] per edge.
        # Layout [128 rows, NT*128 edges] = per-partition contiguous.
        basis = np.concatenate(
            [rbf_pad,
             rhat_pad[:, 0:1] * rbf_pad,
             rhat_pad[:, 1:2] * rbf_pad,
             rhat_pad[:, 2:3] * rbf_pad], axis=1
        )  # [EPAD, 128]
        basisT = np.ascontiguousarray(basis.T)  # [128 rows, EPAD]

        cores.append(dict(srcw=srcw, S=S, basisT=basisT))
    return cores


def _host_weights(W_basis, b_basis, path_w):
    """wext[r]: [128 rows, 11F cols] = [A | RB(3) | RC(3) | D | U(3)].

    Row groups: 0:K = rbf, (j+1)K:(j+2)K = r_j * rbf.
      A   (F)  <- rows 0:K       W*p0     (m0 += a0 * A)
      RB_j(3F) <- rows grp j+1   W*p1     (m0 += av_j * RB_j)
      RC_j(3F) <- rows grp j+1   W*p2     (mv_j += a0 * RC_j)
      D   (F)  <- rows 0:K       W*p3     (mv_j += av_j * D)
      U_j (3F) <- rows grp j+1   W*p4     (cross terms)
    """
    c = CFG
    F = c.F
    Wx = np.zeros((c.R_REF, 4 * c.KB, c.VCOLS), np.float32)
    brep = np.zeros((c.R_REF, 2, F), np.float32)
    for r in range(c.R_REF):
        W = W_basis[r]
        p = path_w[r]
        b = b_basis[r]
        Wx[r, 0:c.KB, c.C_A:c.C_A + F] = W * p[0]
        Wx[r, 0:c.KB, c.C_D:c.C_D + F] = W * p[3]
        for j in range(3):
            rows = slice((j + 1) * c.KB, (j + 2) * c.KB)
            Wx[r, rows, c.C_RB + j * F:c.C_RB + (j + 1) * F] = W * p[1]
            Wx[r, rows, c.C_RC + j * F:c.C_RC + (j + 1) * F] = W * p[2]
            Wx[r, rows, c.C_U + j * F:c.C_U + (j + 1) * F] = W * p[4]
        brep[r, 0] = p[0] * b
        brep[r, 1] = p[3] * b
    return Wx, brep


def _tensor_dense_np(x, W0, W1, b0):
    s = x[:, 0] @ W0 + b0
    v = np.einsum("nif,fg->nig", x[:, 1:4], W1)
    return np.concatenate([s[:, None], v], axis=1)


# ---------------------------------------------------------------- bass build
_BUILD_CACHE = {}


def build(with_bias):
    key = (bool(with_bias), CFG.N_CORES, CFG.NBLK, CFG.TPB, CFG.TSUB)
    if key in _BUILD_CACHE:
        return _BUILD_CACHE[key]

    import concourse.bacc as bacc
    import concourse.mybir as mybir
    import concourse.tile as tile
    from concourse import library_config
    from concourse.alu_op_type import AluOpType
    dt = mybir.dt
    c = CFG
    F = c.F
    ROW = c.ROW

    nc = bacc.Bacc(num_devices=c.N_CORES, target_bir_lowering=False)

    x0f_d = nc.dram_tensor("x0f", [c.NBLK, 128, ROW], dt.float32, kind="ExternalInput")
    xtab0_d = nc.dram_tensor("xtab0", [c.NTAB, ROW], dt.bfloat16, kind="ExternalInput")
    srcw_d = nc.dram_tensor("srcw", [128, c.EPAD // 16], dt.int16, kind="ExternalInput")
    S_d = nc.dram_tensor("sone", [128, c.NT * 128], dt.bfloat16, kind="ExternalInput")
    basis_d = nc.dram_tensor("basisT", [128, c.NT * 128], dt.bfloat16, kind="ExternalInput")
    wext_d = nc.dram_tensor("wext", [c.R_REF, 128, c.VCOLS], dt.bfloat16, kind="ExternalInput")
    brep_d = nc.dram_tensor("brep", [c.R_REF, 2, 128, F], dt.bfloat16, kind="ExternalInput")
    wout_d = nc.dram_tensor("woutrep", [128, ROW], dt.float32, kind="ExternalInput")
    out_d = nc.dram_tensor("out", [c.NBLK, 128, 4], dt.float32, kind="ExternalOutput")

    xtab1_d = nc.dram_tensor("xtab1", [c.NTAB, ROW], dt.bfloat16, addr_space="Shared")
    xtab2_d = nc.dram_tensor("xtab2", [c.NTAB, ROW], dt.bfloat16, addr_space="Shared")
    xslice_d = nc.dram_tensor("xslice", [c.NLOC, ROW], dt.bfloat16)

    tabs = [xtab0_d, xtab1_d, xtab2_d]

    F2 = 2 * F
    F3 = 3 * F
    F4 = 4 * F

    with tile.TileContext(nc) as tc:
        with (
            tc.tile_pool(name="resident", bufs=1) as res,
            tc.tile_pool(name="apool", bufs=3) as apool,
            tc.tile_pool(name="vpsum", bufs=2, space="PSUM") as vpsum,
            tc.tile_pool(name="spsum", bufs=2, space="PSUM") as spsum,
            tc.tile_pool(name="vbfp", bufs=2) as vbfp,
            tc.tile_pool(name="msgp", bufs=2) as msgp,
            tc.tile_pool(name="scr", bufs=2) as scr,
            tc.tile_pool(name="basp", bufs=3) as basp,
            tc.tile_pool(name="sp", bufs=2) as spool,
        ):
            xf32 = res.tile([128, c.NBLK, ROW], dt.float32)
            srcw = res.tile([128, c.EPAD // 16], dt.int16)
            wext = res.tile([128, c.R_REF, c.VCOLS], dt.bfloat16)
            brepA = res.tile([128, c.R_REF, F], dt.bfloat16)
            brepD = res.tile([128, c.R_REF, F], dt.bfloat16)
            woutr = res.tile([128, ROW], dt.float32)
            outsb = res.tile([128, c.NBLK, 4], dt.float32)

            nc.sync.dma_start(xf32[:], x0f_d[:].rearrange("b p f -> p b f"))
            nc.sync.dma_start(srcw[:], srcw_d[:])
            nc.sync.dma_start(wext[:], wext_d[:].rearrange("r p col -> p r col"))
            nc.sync.dma_start(brepA[:], brep_d[:, 0].rearrange("r p f -> p r f"))
            nc.sync.dma_start(brepD[:], brep_d[:, 1].rearrange("r p f -> p r f"))
            nc.sync.dma_start(woutr[:], wout_d[:])

            nc.gpsimd.load_library(library_config.mlp)

            TT = nc.vector.tensor_tensor

            # ---- pipelined job list: one job = one sub-slab ----------
            # per job: gather a, DMA basis, (per tile) proj mm + copy,
            # DVE product chain, (per tile) scatter mm; per block: fold.
            def issue_front(r, b, ss):
                """gather + basis DMA + proj + copy for job (r, b, ss)."""
                tab = tabs[r]
                t0 = b * c.TPB + ss * c.TSUB
                e0 = t0 * 128
                ne = c.TSUB * 128
                a = apool.tile([128, c.TSUB, ROW], dt.bfloat16, tag="a")
                nc.gpsimd.dma_gather(
                    a[:], tab[:],
                    srcw[:, e0 // 16:(e0 + ne) // 16],
                    ne, ne, ROW,
                )
                bas = basp.tile([128, c.TSUB, 128], dt.bfloat16, tag="bas")
                nc.sync.dma_start(
                    bas[:],
                    basis_d[:, e0:e0 + ne].rearrange("p (t e) -> p t e", t=c.TSUB),
                )
                vbf = vbfp.tile([128, c.TSUB, c.VCOLS], dt.bfloat16, tag="vbf")
                for t in range(c.TSUB):
                    vps = vpsum.tile([128, c.VCOLS], dt.float32, tag="vps")
                    for lo in range(0, c.VCOLS, 512):
                        hi = min(lo + 512, c.VCOLS)
                        nc.tensor.matmul(
                            vps[:, lo:hi], bas[:, t, :], wext[:, r, lo:hi],
                            start=True, stop=True,
                        )
                    nc.scalar.activation(
                        vbf[:, t, :], vps[:],
                        mybir.ActivationFunctionType.Copy,
                    )
                if with_bias:
                    TT(vbf[:, :, c.C_A:c.C_A + F],
                       vbf[:, :, c.C_A:c.C_A + F],
                       brepA[:, r, :].unsqueeze(1).to_broadcast([128, c.TSUB, F]),
                       op=AluOpType.add)
                    TT(vbf[:, :, c.C_D:c.C_D + F],
                       vbf[:, :, c.C_D:c.C_D + F],
                       brepD[:, r, :].unsqueeze(1).to_broadcast([128, c.TSUB, F]),
                       op=AluOpType.add)
                return a, vbf

            def issue_mid(a, vbf):
                """DVE product chain -> msg (10F raw product columns)."""
                a0 = a[:, :, 0:F]
                av = a[:, :, F:ROW]
                msg = msgp.tile([128, c.TSUB, c.MCOLS], dt.bfloat16, tag="msg")
                p4 = scr.tile([128, c.TSUB, F4], dt.bfloat16, tag="p4")
                p2 = scr.tile([128, c.TSUB, F2], dt.bfloat16, tag="p2")
                t4 = scr.tile([128, c.TSUB, F3], dt.bfloat16, tag="t4")

                # m0 = sum over the 4-group of a * [A|RB]
                TT(p4[:], a[:, :, 0:F4], vbf[:, :, 0:F4], op=AluOpType.mult)
                TT(p2[:], p4[:, :, 0:F2], p4[:, :, F2:F4], op=AluOpType.add)
                TT(msg[:, :, 0:F], p2[:, :, 0:F], p2[:, :, F:F2],
                   op=AluOpType.add)
                # mv_j = a0 * RC_j + av_j * D
                TT(msg[:, :, c.M_MV:c.M_MV + F3].rearrange(
                       "p t (cc f) -> p t cc f", cc=3),
                   a0.unsqueeze(2).to_broadcast([128, c.TSUB, 3, F]),
                   vbf[:, :, c.C_RC:c.C_RC + F3].rearrange(
                       "p t (cc f) -> p t cc f", cc=3),
                   op=AluOpType.mult)
                TT(t4[:].rearrange("p t (cc f) -> p t cc f", cc=3),
                   av.rearrange("p t (cc f) -> p t cc f", cc=3),
                   vbf[:, :, c.C_D:c.C_D + F].unsqueeze(2).to_broadcast(
                       [128, c.TSUB, 3, F]),
                   op=AluOpType.mult)
                TT(msg[:, :, c.M_MV:c.M_MV + F3],
                   msg[:, :, c.M_MV:c.M_MV + F3], t4[:], op=AluOpType.add)
                # q5_a = av_a * U_{a+1}:  (av0*U1, av1*U2) | av2*U0
                TT(msg[:, :, c.M_Q5:c.M_Q5 + F2],
                   av[:, :, 0:F2], vbf[:, :, c.C_U + F:c.C_U + F3],
                   op=AluOpType.mult)
                TT(msg[:, :, c.M_Q5 + F2:c.M_Q5 + F3],
                   av[:, :, F2:F3], vbf[:, :, c.C_U:c.C_U + F],
                   op=AluOpType.mult)
                # q6_a = av_a * U_{a+2}:  av0*U2 | (av1*U0, av2*U1)
                TT(msg[:, :, c.M_Q6:c.M_Q6 + F],
                   av[:, :, 0:F], vbf[:, :, c.C_U + F2:c.C_U + F3],
                   op=AluOpType.mult)
                TT(msg[:, :, c.M_Q6 + F:c.M_Q6 + F3],
                   av[:, :, F:F3], vbf[:, :, c.C_U:c.C_U + F2],
                   op=AluOpType.mult)
                return msg

            def issue_smm(Sblk, ss, msg, psB):
                """scatter matmuls for the sub-slab into psB."""
                for t in range(c.TSUB):
                    tt = ss * c.TSUB + t
                    first = tt == 0
                    last = tt == c.TPB - 1
                    for lo in range(0, c.MCOLS, 512):
                        hi = min(lo + 512, c.MCOLS)
                        nc.tensor.matmul(
                            psB[:, lo:hi], Sblk[:, tt, :], msg[:, t, lo:hi],
                            start=first, stop=last,
                        )

            def issue_fold(b, psB):
                """psB (640 fp32) -> xf32 with cross-term rotation."""
                xb = xf32[:, b, :]
                # x0 += m0
                TT(xb[:, 0:F], xb[:, 0:F], psB[:, 0:F], op=AluOpType.add)
                # xv += mv
                TT(xb[:, F:F4], xb[:, F:F4], psB[:, c.M_MV:c.M_MV + F3],
                   op=AluOpType.add)
                # xv_i += q5_{(i+1)%3}
                TT(xb[:, F:F3], xb[:, F:F3],
                   psB[:, c.M_Q5 + F:c.M_Q5 + F3], op=AluOpType.add)
                TT(xb[:, F3:F4], xb[:, F3:F4],
                   psB[:, c.M_Q5:c.M_Q5 + F], op=AluOpType.add)
                # xv_i -= q6_{(i+2)%3}
                TT(xb[:, F2:F4], xb[:, F2:F4],
                   psB[:, c.M_Q6:c.M_Q6 + F2], op=AluOpType.subtract)
                TT(xb[:, F:F2], xb[:, F:F2],
                   psB[:, c.M_Q6 + F2:c.M_Q6 + F3], op=AluOpType.subtract)

            # flat job list over (r, b, ss)
            jobs = [(r, b, ss)
                    for r in range(c.R_REF)
                    for b in range(c.NBLK)
                    for ss in range(c.NSUB)]

            def load_S(b):
                Sblk = spool.tile([128, c.TPB, 128], dt.bfloat16, tag="S")
                nc.sync.dma_start(
                    Sblk[:],
                    S_d[:, b * c.EPB:(b + 1) * c.EPB].rearrange(
                        "p (t e) -> p t e", t=c.TPB),
                )
                return Sblk

            # software pipeline: front(j+1) is issued before smm(j) so the
            # PE's in-order queue runs proj(j+1) ahead of the barrier-ing
            # scatter matmul of job j.  At a refinement boundary front(j+1)
            # must instead be issued after the AllGather (it reads the new
            # table), breaking the pipeline for that one job.
            state = {}  # j -> (a, vbf)
            Scur = load_S(jobs[0][1])
            pending_S = None
            state[0] = issue_front(*jobs[0])
            psB = spsum.tile([128, c.MCOLS], dt.float32, tag="psB")
            for j, (r, b, ss) in enumerate(jobs):
                nxt = jobs[j + 1] if j + 1 < len(jobs) else None
                boundary = nxt is not None and nxt[0] != r
                if nxt is not None and not boundary:
                    state[j + 1] = issue_front(*nxt)
                    if nxt[1] != b:
                        pending_S = load_S(nxt[1])
                a, vbf = state.pop(j)
                msg = issue_mid(a, vbf)
                issue_smm(Scur, ss, msg, psB)
                if ss == c.NSUB - 1:  # block end
                    issue_fold(b, psB)
                    if nxt is not None:
                        psB = spsum.tile([128, c.MCOLS], dt.float32,
                                         tag="psB")
                        if boundary:
                            nc.gpsimd.dma_start(
                                xslice_d[:].rearrange("(b p) f -> p b f",
                                                      p=128),
                                xf32[:],
                            )
                            nc.gpsimd.collective_compute(
                                "AllGather",
                                mybir.AluOpType.bypass,
                                replica_groups=[list(range(c.N_CORES))],
                                ins=[xslice_d[:]],
                                outs=[tabs[r + 1][:]],
                            )
                            state[j + 1] = issue_front(*nxt)
                            pending_S = load_S(nxt[1])
                        if pending_S is not None:
                            Scur = pending_S
                            pending_S = None

            for b in range(c.NBLK):
                tq = scr.tile([128, ROW], dt.float32, tag="tq")
                TT(tq[:], xf32[:, b, :], woutr[:], op=AluOpType.mult)
                nc.vector.tensor_reduce(
                    outsb[:, b, :].unsqueeze(-1),
                    tq[:].rearrange("p (l f) -> p l f", l=4),
                    axis=mybir.AxisListType.X,
                    op=AluOpType.add,
                )
            nc.sync.dma_start(out_d[:].rearrange("b p l -> p b l"), outsb[:])

    nc.compile()
    _BUILD_CACHE[key] = nc
    return nc


# ---------------------------------------------------------------- runner
def prep_in_maps(inputs):
    """Host preprocessing -> (in_maps, with_bias)."""
    import ml_dtypes
    c = CFG
    bf16 = ml_dtypes.bfloat16

    x_dftb = np.asarray(inputs["x_dftb"], np.float32)
    coords = np.asarray(inputs["coords"], np.float32)
    dst_idx = np.asarray(inputs["dst_idx"], np.int32)
    src_idx = np.asarray(inputs["src_idx"], np.int32)
    W_in0 = np.asarray(inputs["W_in0"], np.float32)
    W_in1 = np.asarray(inputs["W_in1"], np.float32)
    b_in = np.asarray(inputs["b_in"], np.float32)
    W_basis = np.asarray(inputs["W_basis"], np.float32)
    b_basis = np.asarray(inputs["b_basis"], np.float32)
    path_w = np.asarray(inputs["path_w"], np.float32)
    W_out0 = np.asarray(inputs["W_out0"], np.float32)
    W_out1 = np.asarray(inputs["W_out1"], np.float32)

    rhat, rbf = _host_geometry(coords, dst_idx, src_idx)
    shards = _host_shard(dst_idx, src_idx, rhat, rbf)
    Wx, brep = _host_weights(W_basis, b_basis, path_w)

    x0 = _tensor_dense_np(x_dftb, W_in0, W_in1, b_in)
    x0_rows = x0.reshape(c.N_NODES, c.ROW)

    tab0 = np.zeros((c.NTAB, c.ROW), np.float32)
    x0fs = []
    for ci in range(c.N_CORES):
        x0c = np.zeros((c.NLOC, c.ROW), np.float32)
        x0c[:c.NPC] = x0_rows[ci * c.NPC:(ci + 1) * c.NPC]
        tab0[ci * c.TAB_STRIDE:ci * c.TAB_STRIDE + c.NLOC] = x0c
        x0fs.append(x0c)

    wout = np.concatenate(
        [W_out0[:, 0], W_out1[:, 0], W_out1[:, 0], W_out1[:, 0]]
    ).astype(np.float32)
    wout_rep = np.ascontiguousarray(
        np.broadcast_to(wout[None, :], (128, c.ROW)))

    tab0_bf = tab0.astype(bf16)
    wext_bf = Wx.astype(bf16)
    brep_bf = np.ascontiguousarray(
        np.broadcast_to(brep[:, :, None, :], (c.R_REF, 2, 128, c.F))
    ).astype(bf16)

    in_maps = []
    for ci in range(c.N_CORES):
        sh = shards[ci]
        in_maps.append(dict(
            x0f=np.ascontiguousarray(x0fs[ci].reshape(c.NBLK, 128, c.ROW)),
            xtab0=tab0_bf,
            srcw=sh["srcw"],
            sone=sh["S"].astype(bf16),
            basisT=sh["basisT"].astype(bf16),
            wext=wext_bf,
            brep=brep_bf,
            woutrep=wout_rep,
        ))
    with_bias = bool(np.any(b_basis != 0.0))
    return in_maps, with_bias


def assemble_output(results, b_out):
    c = CFG
    outs = []
    for ci in range(c.N_CORES):
        o = np.asarray(results[ci]["out"]).reshape(c.NLOC, 4)[:c.NPC]
        outs.append(o)
    out = np.concatenate(outs, axis=0).reshape(c.N_NODES, 4, 1)
    out = out.astype(np.float32).copy()
    out[:, 0, :] += np.asarray(b_out, np.float32)[None, :]
    return out


LAST_RESULTS = None


def kernel(**inputs):
    global LAST_RESULTS
    from concourse.bass_utils import run_bass_kernel_spmd

    in_maps, with_bias = prep_in_maps(inputs)
    nc = build(with_bias)
    trace = bool(os.environ.get("KERNEL_TRACE"))
    res = run_bass_kernel_spmd(
        nc, in_maps, core_ids=list(range(CFG.N_CORES)), trace=trace)
    LAST_RESULTS = res
    return assemble_output(res.results, inputs["b_out"])
